# revision 1
# baseline (speedup 1.0000x reference)
"""DecoderLSTM Trainium2 kernel.

Problem: N=32768 batch rows, H=512 hidden, O=2 outputs, T=12 steps.
    h,c,x = (context, 0, start_pos)
    per step: gates = x @ W_ih.T + h @ W_hh.T + (b_ih + b_hh)
              i,f,g,o = split(gates); c = sig(f)*c + sig(i)*tanh(g)
              h = sig(o)*tanh(c); pred = h @ W_fc.T + b_fc; x = pred
    output: preds stacked [N, T, O]

Key algebraic fold: x_t = W_fc @ h_t + b_fc for t>=1, so
    gates_t = (W_hh + W_ih W_fc) @ h_{t-1} + (b + W_ih b_fc)   (t >= 1)
    gates_0 = W_hh @ context + W_ih @ start_pos + b
which removes the fed-back x input entirely (rank-2 weight correction,
done once on the host).

Sharding: pure data parallel, 4096 batch rows per core, weights replicated.

On-chip layout is "transposed": H lives on partitions (4 chunks of 128),
batch on the free dim, so the recurrence needs no transposes at all:
    gates.T[4H, B] = W_eff @ h.T  via matmul(lhsT=W_eff.T chunk, rhs=h.T chunk)
Per-gate bias is applied for free via the ACT engine's per-partition bias
on the sigmoid/tanh evacuation of PSUM.

Each core processes its 4096 batch cols as 4 chunks of 1024; two chunks
("chains") are interleaved so one chain's matmuls hide the other chain's
activation/elementwise tail between steps.
"""

import os

import numpy as np

import concourse.mybir as mybir
from concourse import bacc
import concourse.tile as tile
from concourse.bass_utils import run_bass_kernel_spmd

P = 128
H = 512
HC = H // P          # 4 H-chunks
G4 = 4 * H           # 2048 gate rows
O = 2
T = 12
N_FULL = 32768
N_CORES = 8
NB = N_FULL // N_CORES   # 4096 batch rows per core
BC = 1024                # batch cols per chain
NCH = NB // BC           # 4 chains per core
F32 = mybir.dt.float32
# float16 runs the PE at full rate (1 col/cycle) with overlapped FWL weight
# loads, and its 10-bit mantissa keeps the recurrence error ~4e-4 of scale.
# float32r (same speed on paper) measured ~20% slower on HW: its self-loading
# weight path serializes against the matmul stream. float32 is exact but 4x.
_MM_DT_NAME = os.environ.get("KERNEL_MM_DT", "float16")
MM_DT = getattr(mybir.dt, _MM_DT_NAME)

# Moving-operand free dim per matmul: one 512-entry PSUM bank.
MMN = 512
NHALF = BC // MMN

# Buffer depths (SBUF is ample at fp16): h ping-pong depth also controls how
# early the next pair's h0 DMA can start; act tiles decouple ACT from DVE.
H_BUFS = int(os.environ.get("KERNEL_HBUFS", "2"))
A_BUFS = int(os.environ.get("KERNEL_ABUFS", "2"))

SIG = mybir.ActivationFunctionType.Sigmoid
TANH = mybir.ActivationFunctionType.Tanh


def build_bass():
    # Bacc (not plain Bass): its compile() runs move_matmul_waits_to_ldweights
    # + generate_event_semaphores, which split semaphore waits to the 1-per-
    # instruction hardware limit. Without it, walrus rejects any 2-dep
    # instruction with 'Too many sync wait commands'.
    nc = bacc.Bacc()

    # Tensors feeding the PE are declared in MM_DT end-to-end (for fp32r
    # the BIR verifier additionally requires matmul operands to be *produced*
    # as fp32r, i.e. rounded at the producer).
    h0_d = nc.dram_tensor("h0", [P, HC, NB], MM_DT, kind="ExternalInput")
    sp_d = nc.dram_tensor("sp", [P, NB], MM_DT, kind="ExternalInput")
    wg_d = nc.dram_tensor("wg", [P, HC, G4], MM_DT, kind="ExternalInput")
    wx_d = nc.dram_tensor("wx", [P, G4], MM_DT, kind="ExternalInput")
    wf_d = nc.dram_tensor("wf", [P, HC, O], MM_DT, kind="ExternalInput")
    b0_d = nc.dram_tensor("b0", [P, G4 // P], F32, kind="ExternalInput")
    b1_d = nc.dram_tensor("b1", [P, G4 // P], F32, kind="ExternalInput")
    bfc_d = nc.dram_tensor("bfc", [O, 1], F32, kind="ExternalInput")
    out_d = nc.dram_tensor("preds", [2 * T, NB], F32, kind="ExternalOutput")

    with tile.TileContext(nc) as tc:
        with (
            tc.tile_pool(name="singles", bufs=1) as singles,
            tc.tile_pool(name="state", bufs=1) as state,
            tc.tile_pool(name="acts", bufs=2) as acts,
            tc.tile_pool(name="gpsum", bufs=3, space="PSUM") as gpsum,
            tc.tile_pool(name="ppsum", bufs=1, space="PSUM") as ppsum,
        ):
            wg_sb = singles.tile([P, HC, G4], MM_DT)
            # split by K-chunk so the first matmuls start after ~1/4 of the
            # weights have landed (the only PE idle in the modeled trace is
            # ~16us at startup waiting for these loads)
            for _kj in range(HC):
                nc.sync.dma_start(wg_sb[:, _kj, :], wg_d[:, _kj, :])
            wx_sb = singles.tile([P, G4], MM_DT)
            nc.sync.dma_start(wx_sb[:], wx_d[:])
            wf_sb = singles.tile([P, HC, O], MM_DT)
            nc.sync.dma_start(wf_sb[:], wf_d[:])
            sp_sb = singles.tile([P, NB], MM_DT)
            nc.sync.dma_start(sp_sb[:], sp_d[:])
            b0_sb = singles.tile([P, G4 // P], F32)
            nc.sync.dma_start(b0_sb[:], b0_d[:])
            b1_sb = singles.tile([P, G4 // P], F32)
            nc.sync.dma_start(b1_sb[:], b1_d[:])
            bfc_sb = singles.tile([O, 1], F32)
            nc.sync.dma_start(bfc_sb[:], bfc_d[:])

            st = {}

            def emit_step(s, t):
                """Gates + elementwise for chain s, step t. Updates st[s]."""
                h_prev, c = st[s]
                h_next = state.tile([P, HC, BC], MM_DT, tag=f"h{s % 2}",
                                    bufs=H_BUFS)
                b_sb = b0_sb if t == 0 else b1_sb
                for j in range(HC):
                    gsb = []
                    for g in range(4):
                        mb = 4 * g + j
                        msl = slice(mb * P, (mb + 1) * P)
                        gp = gpsum.tile([P, BC], F32, tag="g")
                        for half in range(NHALF):
                            cs = slice(half * MMN, (half + 1) * MMN)
                            for kj in range(HC):
                                nc.tensor.matmul(
                                    gp[:, cs],
                                    wg_sb[:, kj, msl],
                                    h_prev[:, kj, cs],
                                    start=(kj == 0),
                                    stop=(kj == HC - 1 and t > 0),
                                )
                            if t == 0:
                                scs = slice(s * BC + half * MMN,
                                            s * BC + (half + 1) * MMN)
                                nc.tensor.matmul(
                                    gp[:, cs],
                                    wx_sb[:, msl],
                                    sp_sb[:, scs],
                                    start=False,
                                    stop=True,
                                )
                        a = acts.tile([P, BC], F32, tag=f"a{g}", bufs=A_BUFS)
                        nc.scalar.activation(
                            a[:], gp[:],
                            TANH if g == 2 else SIG,
                            bias=b_sb[:, mb:mb + 1],
                        )
                        gsb.append(a)
                    ai, af, ag, ao = gsb
                    # c_j = f * c_j + i * g ; h_j = o * tanh(c_j)
                    nc.vector.tensor_mul(ai[:], ai[:], ag[:])          # i*g -> ai
                    nc.vector.tensor_mul(c[:, j, :], af[:], c[:, j, :])
                    nc.vector.tensor_add(c[:, j, :], c[:, j, :], ai[:])
                    nc.scalar.activation(ag[:], c[:, j, :], TANH)      # tanh(c) -> ag
                    nc.vector.tensor_mul(h_next[:, j, :], ao[:], ag[:])
                st[s] = (h_next, c)

            def emit_pred(s, t):
                """pred_t = W_fc @ h.T + b_fc -> DRAM rows [2t, 2t+2)."""
                h_cur = st[s][0]
                pp = ppsum.tile([O, NHALF, MMN], F32, tag="pred")
                for half in range(NHALF):
                    cs = slice(half * MMN, (half + 1) * MMN)
                    for kj in range(HC):
                        nc.tensor.matmul(
                            pp[:, half, :],
                            wf_sb[:, kj, :],
                            h_cur[:, kj, cs],
                            start=(kj == 0),
                            stop=(kj == HC - 1),
                        )
                psb = acts.tile([O, NHALF, MMN], F32, tag="pred_sb", bufs=1)
                nc.vector.tensor_scalar_add(psb[:], pp[:], bfc_sb[:, 0:1])
                dst = out_d[2 * t:2 * t + 2, s * BC:(s + 1) * BC]
                nc.sync.dma_start(
                    dst.rearrange("p (h x) -> p h x", h=NHALF), psb[:])

            for pair in range(NCH // 2):
                chains = [2 * pair, 2 * pair + 1]
                for s in chains:
                    h = state.tile([P, HC, BC], MM_DT, tag=f"h{s % 2}",
                                   bufs=H_BUFS)
                    for _kj in range(HC):
                        nc.sync.dma_start(
                            h[:, _kj, :],
                            h0_d[:, _kj, s * BC:(s + 1) * BC])
                    c = state.tile([P, HC, BC], F32, tag=f"c{s % 2}", bufs=1)
                    nc.vector.memset(c[:], 0.0)
                    st[s] = (h, c)
                a, b = chains
                for t in range(T):
                    emit_step(a, t)
                    if t > 0:
                        emit_pred(b, t - 1)
                    emit_step(b, t)
                    emit_pred(a, t)
                emit_pred(b, T - 1)

    nc.compile()
    return nc


_NC_CACHE = {}


def _get_nc():
    key = _MM_DT_NAME
    if key not in _NC_CACHE:
        _NC_CACHE[key] = build_bass()
    return _NC_CACHE[key]


MM_NP = mybir.dt.np(MM_DT)


def prepare_in_maps(inputs):
    ctx = np.ascontiguousarray(np.asarray(inputs["context"], dtype=np.float32))
    sp = np.ascontiguousarray(np.asarray(inputs["start_pos"], dtype=np.float32))
    W_ih = np.asarray(inputs["W_ih"], dtype=np.float32)
    W_hh = np.asarray(inputs["W_hh"], dtype=np.float32)
    b_ih = np.asarray(inputs["b_ih"], dtype=np.float32)
    b_hh = np.asarray(inputs["b_hh"], dtype=np.float32)
    W_fc = np.asarray(inputs["W_fc"], dtype=np.float32)
    b_fc = np.asarray(inputs["b_fc"], dtype=np.float32)

    # Fold the fed-back fc layer into the recurrence (exact algebra; done in
    # fp64 to keep the fold itself error-free).
    W_eff = (W_hh.astype(np.float64) @ np.eye(H)
             + W_ih.astype(np.float64) @ W_fc.astype(np.float64)).astype(np.float32)
    b0 = (b_ih.astype(np.float64) + b_hh.astype(np.float64)).astype(np.float32)
    b1 = (b_ih.astype(np.float64) + b_hh.astype(np.float64)
          + W_ih.astype(np.float64) @ b_fc.astype(np.float64)).astype(np.float32)

    # SBUF layouts: partition dim = H chunk of 128.
    wg = np.ascontiguousarray(
        W_eff.T.reshape(HC, P, G4).transpose(1, 0, 2)).astype(MM_NP)
    wx = np.zeros((P, G4), MM_NP)
    wx[:O] = W_ih.T.astype(MM_NP)                               # K-padded to 128
    wf = np.ascontiguousarray(
        W_fc.T.reshape(HC, P, O).transpose(1, 0, 2)).astype(MM_NP)
    b0s = np.ascontiguousarray(b0.reshape(G4 // P, P).T)        # [128, 16]
    b1s = np.ascontiguousarray(b1.reshape(G4 // P, P).T)
    bfc = np.ascontiguousarray(b_fc.reshape(O, 1))

    in_maps = []
    for core in range(N_CORES):
        sl = slice(core * NB, (core + 1) * NB)
        h0 = np.ascontiguousarray(
            ctx[sl].T.reshape(HC, P, NB).transpose(1, 0, 2)).astype(MM_NP)
        spc = np.zeros((P, NB), MM_NP)
        # The device uses W_eff for step 0 too, which adds a spurious
        # W_ih@W_fc@context term; feeding sp' = start_pos - context@W_fc.T
        # cancels it exactly (step 0 is linear in its x input).
        sp_eff = (sp[sl].astype(np.float64)
                  - ctx[sl].astype(np.float64) @ W_fc.astype(np.float64).T)
        spc[:O] = sp_eff.T.astype(MM_NP)                        # K-padded to 128
        in_maps.append({
            "h0": h0, "sp": spc, "wg": wg, "wx": wx, "wf": wf,
            "b0": b0s, "b1": b1s, "bfc": bfc,
        })
    return in_maps


def assemble_output(results):
    out = np.empty((N_FULL, T, O), np.float32)
    for core in range(N_CORES):
        pr = results[core]["preds"]                             # [24, 4096]
        out[core * NB:(core + 1) * NB] = (
            pr.reshape(T, O, NB).transpose(2, 0, 1))
    return out


def kernel(**inputs):
    in_maps = prepare_in_maps(inputs)
    nc = _get_nc()
    res = run_bass_kernel_spmd(
        nc, in_maps, core_ids=list(range(N_CORES)), trace=False,
    )
    if res.exec_time_ns is not None:
        kernel.last_exec_time_ns = res.exec_time_ns
    return assemble_output(res.results)

